# revision 25
# baseline (speedup 1.0000x reference)
"""Trainium2 Bass kernel: cross-entropy with Gaussian-smoothed labels.

loss = mean over tokens of  [ Wsum(t) * logsumexp(pred_row) - sum_k w_k * pred[start+k] ]

where the smoothed one-hot reduces exactly to a 7-tap window:
  start = clip(t-3, 0, C-7), u = t-start, w_k = f(k-u)
  f(0)=1.0, f(+-m)=exp(-2^m/4) for m in 1..3, else 0.

Sharding: pure data-parallel over the batch axis, 4 batches (8192 tokens)
per core across 8 cores. Per core:
  - stream pred [8192, 722] f32 through SBUF in [128, G*722] tiles;
    one ACT-engine Exp per group; sum(exp) via one DVE reduce per group
    (a few groups use the ACT accumulator instead, to balance engines).
  - 64 indirect DMAs gather the 7-wide windows (one offset per partition;
    larger offset tables are not honored by the SWDGE ucode).
  - weights built once on-chip from iota/compares ([128, 64, 7]).
  - the weighted-gather dot is LAST in DVE program order so its
    dependency on all 64 gathers cannot head-of-line-block the DVE queue.
  - per-core partial sums [128, 1] DMA'd out; host sums 8x128 and divides.
"""
import math

import numpy as np

import concourse.bass as bass
import concourse.bacc as bacc
import concourse.tile as tile
from concourse import mybir
from concourse import bass_utils

B, T, C = 32, 2048, 722
CORES = 8
SHARD = B * T // CORES          # 8192 tokens per core
P = 128
TILES = SHARD // P              # 64 tokens per partition
K = 7
START_MAX = C - K               # 715
DECAYS = [math.exp(-(2.0 ** d) / 4.0) for d in range(4)]

_ALU = mybir.AluOpType
_ACT = mybir.ActivationFunctionType

_NC = None
# Tunables (overridable by test.py before first _run call).
CFG = dict(G=4, act_groups=(2, 5, 8, 11, 14), pred_bufs=3, exp_bufs=3,
           swdge_queues=1, single_packet=True)


def _bcast_inner(ap, n):
    """Append a step-0 broadcast dim of length n to an AP."""
    return bass.AP(tensor=ap.tensor, offset=ap.offset, ap=[*ap.ap, [0, n]])


def _build(G=4, act_groups=(), pred_bufs=3, exp_bufs=3, swdge_queues=1,
           single_packet=False):
    nc = bacc.Bacc("TRN2", target_bir_lowering=False, debug=False,
                   enable_asserts=True, num_devices=CORES,
                   num_swdge_queues=swdge_queues)
    pred = nc.dram_tensor("pred", [SHARD, C], mybir.dt.float32, kind="ExternalInput")
    target = nc.dram_tensor("target", [SHARD], mybir.dt.int32, kind="ExternalInput")
    # pure constants fed as inputs so gpsimd never runs iota (its queue is
    # 100% busy with the 64 gather calls — the kernel's critical path)
    c_row = nc.dram_tensor("c_row", [P, TILES], mybir.dt.int32, kind="ExternalInput")
    c_iok = nc.dram_tensor("c_iok", [P, TILES, K], mybir.dt.float32,
                           kind="ExternalInput")
    c_one = nc.dram_tensor("c_one", [P, 1], mybir.dt.float32, kind="ExternalInput")
    out = nc.dram_tensor("partial", [1, 1], mybir.dt.float32, kind="ExternalOutput")

    pred_flat = pred.ap().rearrange("a b -> (a b)").rearrange("(n one) -> n one", one=1)
    # token index = p*TILES + j*G + g  (each partition owns a contiguous slab)
    pred_g = pred.ap().rearrange("(p j g) c -> j p g c", p=P, g=G)
    NG = TILES // G

    with tile.TileContext(nc) as tc:
        with (tc.tile_pool(name="pred", bufs=pred_bufs) as pred_pool,
              tc.tile_pool(name="exp", bufs=exp_bufs) as exp_pool,
              tc.tile_pool(name="small", bufs=1) as small,
              tc.tile_pool(name="psum", bufs=1, space="PSUM") as psump):
            # targets: tgt_sb[p, j] = target[p*TILES + j].  The whole offset
            # chain runs at high priority: every gather depends on it, and
            # the 64-call SWDGE sequence is the kernel's critical path.
            with tc.high_priority():
                tgt_sb = small.tile([P, TILES], mybir.dt.int32)
                nc.sync.dma_start(out=tgt_sb,
                                  in_=target.ap().rearrange("(p j) -> p j", p=P))

                # flat element offsets of each token's window start
                row = small.tile([P, TILES], mybir.dt.int32)
                nc.sync.dma_start(out=row, in_=c_row.ap())
                start_i = small.tile([P, TILES], mybir.dt.int32)
                nc.vector.tensor_scalar(out=start_i, in0=tgt_sb, scalar1=3,
                                        scalar2=0, op0=_ALU.subtract, op1=_ALU.max)
                nc.vector.tensor_scalar_min(out=start_i, in0=start_i,
                                            scalar1=START_MAX)
                offs = small.tile([P, TILES], mybir.dt.int32)
                nc.vector.tensor_scalar_mul(out=offs, in0=row, scalar1=C)
                nc.vector.tensor_add(out=offs, in0=offs, in1=start_i)
            ones = small.tile([P, 1], mybir.dt.float32)
            nc.sync.dma_start(out=ones, in_=c_one.ap())

            # windowed gathers: one indirect DMA per token-tile ([128, 7] out,
            # [128, 1] offsets — the only shape the SWDGE ucode honors).
            # Alternate between SWDGE queues so completion waits overlap.
            gath = small.tile([P, TILES, K], mybir.dt.float32)
            orig_dmacopy = mybir.InstDMACopy
            for j in range(TILES):
                kw_extra = {}
                if swdge_queues > 1:
                    kw_extra["queue"] = f"qPoolDynamic{j % swdge_queues or ''}"
                if single_packet:
                    kw_extra["single_packet"] = True
                if kw_extra:
                    def _patched(*a, _kw=kw_extra, **kw):
                        kw.update(_kw)
                        return orig_dmacopy(*a, **kw)

                    mybir.InstDMACopy = _patched
                try:
                    nc.gpsimd.indirect_dma_start(
                        out=gath[:, j, :],
                        out_offset=None,
                        in_=pred_flat,
                        in_offset=bass.IndirectOffsetOnAxis(
                            ap=offs[:, j:j + 1], axis=0),
                    )
                finally:
                    mybir.InstDMACopy = orig_dmacopy

            # u = t - start in f32 (0..6); diff[p,j,k] = k - u[p,j]
            tf = small.tile([P, TILES], mybir.dt.float32)
            nc.vector.tensor_copy(out=tf, in_=tgt_sb)
            sf = small.tile([P, TILES], mybir.dt.float32)
            nc.vector.tensor_copy(out=sf, in_=start_i)
            uf = small.tile([P, TILES], mybir.dt.float32)
            nc.vector.tensor_sub(out=uf, in0=tf, in1=sf)

            iok = small.tile([P, TILES, K], mybir.dt.float32)
            nc.sync.dma_start(out=iok, in_=c_iok.ap())
            diff = small.tile([P, TILES, K], mybir.dt.float32)
            nc.vector.scalar_tensor_tensor(out=diff, in0=iok, scalar=1.0,
                                           in1=_bcast_inner(uf, K),
                                           op0=_ALU.mult, op1=_ALU.subtract)
            # w = 1.0*(diff==0) + sum_m DECAYS[m]*(|diff|==m)
            w = small.tile([P, TILES, K], mybir.dt.float32)
            nc.vector.tensor_scalar(out=w, in0=diff, scalar1=0.0, scalar2=None,
                                    op0=_ALU.is_equal)
            tmp = small.tile([P, TILES, K], mybir.dt.float32)
            for m in (1, 2, 3):
                for s in (-m, m):
                    nc.vector.tensor_scalar(out=tmp, in0=diff, scalar1=float(s),
                                            scalar2=None, op0=_ALU.is_equal)
                    nc.vector.scalar_tensor_tensor(out=w, in0=tmp, scalar=DECAYS[m],
                                                   in1=w, op0=_ALU.mult, op1=_ALU.add)
            wsum = small.tile([P, TILES], mybir.dt.float32)
            nc.vector.reduce_sum(out=wsum, in_=w, axis=mybir.AxisListType.X)

            # dense stream: exp per group, reduce -> sum(exp) per token
            sums = small.tile([P, TILES], mybir.dt.float32)
            for jg in range(NG):
                pt = pred_pool.tile([P, G, C], mybir.dt.float32)
                nc.sync.dma_start(out=pt, in_=pred_g[jg])
                if jg in act_groups:
                    # per-token accumulate on the ACT engine
                    for g in range(G):
                        j = jg * G + g
                        et = exp_pool.tile([P, C], mybir.dt.float32, tag="et_acc")
                        nc.scalar.activation(out=et, in_=pt[:, g, :], func=_ACT.Exp,
                                             accum_out=sums[:, j:j + 1])
                else:
                    et = exp_pool.tile([P, G, C], mybir.dt.float32, tag="et_dve")
                    nc.scalar.activation(out=et, in_=pt, func=_ACT.Exp)
                    nc.vector.reduce_sum(out=sums[:, jg * G:(jg + 1) * G], in_=et,
                                         axis=mybir.AxisListType.X)

            # tail: weighted gather dot + lse + loss. Scheduled past the end
            # of everything else (tile_wait_until affects only the
            # scheduler's placement, not runtime semaphores) so its
            # dependency on all 64 gathers can't head-of-line-block the DVE
            # queue mid-stream.
            with tc.tile_wait_until(0.2):
                wg = small.tile([P, TILES, K], mybir.dt.float32)
                nc.vector.tensor_mul(out=wg, in0=w, in1=gath)
                gsum = small.tile([P, TILES], mybir.dt.float32)
                nc.vector.reduce_sum(out=gsum, in_=wg, axis=mybir.AxisListType.X)

                lse = small.tile([P, TILES], mybir.dt.float32)
                nc.scalar.activation(out=lse, in_=sums, func=_ACT.Ln)
                loss = small.tile([P, TILES], mybir.dt.float32)
                nc.vector.tensor_mul(out=loss, in0=wsum, in1=lse)
                nc.vector.tensor_sub(out=loss, in0=loss, in1=gsum)
                part = small.tile([P, 1], mybir.dt.float32)
                nc.vector.reduce_sum(out=part, in_=loss, axis=mybir.AxisListType.X)
            # cross-partition reduce on the (idle) PE: a [128,1] output DMA
            # is 128 four-byte descriptors whose completion semaphores
            # serialize ~400ns apart (~7us of teardown); a [1,1] scalar
            # output is one descriptor.
            ps = psump.tile([1, 1], mybir.dt.float32)
            nc.tensor.matmul(out=ps, lhsT=part, rhs=ones, start=True, stop=True)
            total = small.tile([1, 1], mybir.dt.float32)
            nc.vector.tensor_copy(out=total, in_=ps)
            nc.sync.dma_start(out=out.ap(), in_=total)
    nc.compile()
    return nc


def _get_nc():
    global _NC
    if _NC is None:
        _NC = _build(**CFG)
    return _NC


_C_ROW = (np.arange(SHARD, dtype=np.int32).reshape(P, TILES))
_C_IOK = np.broadcast_to(
    np.arange(K, dtype=np.float32), (P, TILES, K)).copy()
_C_ONE = np.ones((P, 1), dtype=np.float32)


def _shard_inputs(pred, target):
    bpc = B // CORES
    in_maps = []
    for c in range(CORES):
        in_maps.append({
            "pred": np.ascontiguousarray(
                pred[c * bpc:(c + 1) * bpc].reshape(SHARD, C), dtype=np.float32),
            "target": np.ascontiguousarray(
                target[c * bpc:(c + 1) * bpc].reshape(SHARD), dtype=np.int32),
            "c_row": _C_ROW,
            "c_iok": _C_IOK,
            "c_one": _C_ONE,
        })
    return in_maps


def _run(pred, target, **kwargs):
    nc = _get_nc()
    return bass_utils.run_bass_kernel_spmd(
        nc, _shard_inputs(pred, target), core_ids=list(range(CORES)), **kwargs)


def kernel(pred, target):
    res = _run(pred, target)
    total = sum(float(r["partial"].astype(np.float64).sum()) for r in res.results)
    return np.asarray(total / (B * T), dtype=np.float32)
